# revision 1
# baseline (speedup 1.0000x reference)
"""Dot-product stereo cost volume on 8 Trainium2 NeuronCores.

cost[b, d, y, x] = sum_c left[b,c,y,x] * right[b,c,y,x-d], zeros where x-d < 0.
Shapes: left/right [4, 128, 192, 640] fp32, D = 96 -> out [4, 96, 192, 640] fp32.

Strategy
--------
Sharding: 8 cores <- (b, y-half): core k handles batch k//2, rows 96*(k%2)..+96.
No halo needed (disparity shifts are along W only).

Per (y) row the math is a banded Gram matrix: G_y[x', x] = sum_c R[c,x'] L[c,x],
and cost[d, y, x] = G_y[x-d, x].  The PE computes G in 32-row tiles:
tile t covers x' in [32t, 32t+32), x in [32t, 32t+128) (since d <= 95, every
needed (x', x) pair with x' in that 32-block satisfies 0 <= x - x' <= 127).
Four such M=32 matmuls run concurrently in the 128-wide PE array via
tile_position column groups, stacked into one [128, 128] PSUM tile.  The raw
rect tiles stream to a DRAM scratch buffer; the diagonal reindex (d = x - x')
is absorbed into the host-side unshard with one precomputed fancy index.
"""

import sys

if "/opt/trn_rl_repo" not in sys.path:
    sys.path.insert(0, "/opt/trn_rl_repo")

import numpy as np

B, C, H, W = 4, 128, 192, 640
D = 96
HSH = H // 2          # rows per core
MT = 32               # M (x') tile height
FT = 128              # free (x) tile width
NT = W // MT          # 20 x'-tiles per row
NG = NT // 4          # 5 groups of 4 col-tiled matmuls

_compiled = None


def _build():
    import concourse.bacc as bacc
    import concourse.tile as tile
    import concourse.mybir as mybir

    nc = bacc.Bacc("TRN2", target_bir_lowering=False, debug=False, num_devices=8)
    f32 = mybir.dt.float32

    left_ap = nc.dram_tensor("left", [C, HSH, W], f32, kind="ExternalInput").ap()
    right_ap = nc.dram_tensor("right", [C, HSH, W], f32, kind="ExternalInput").ap()
    scr_ap = nc.dram_tensor("scr", [HSH, NG, 128, FT], f32, kind="ExternalOutput").ap()

    WPAD = W + FT  # L is zero-padded on the right so every rhs window is full

    with tile.TileContext(nc) as tc:
        with (
            tc.tile_pool(name="lpool", bufs=3) as lpool,
            tc.tile_pool(name="rpool", bufs=3) as rpool,
            tc.tile_pool(name="stage", bufs=4) as stage_pool,
            tc.tile_pool(name="psum", bufs=4, space="PSUM") as psum_pool,
        ):
            for y in range(HSH):
                lt = lpool.tile([128, WPAD], f32, name=f"lt_{y}", tag="lt")
                rt = rpool.tile([128, W], f32, name=f"rt_{y}", tag="rt")
                nc.sync.dma_start(lt[:, 0:W], left_ap[:, y, :])
                nc.vector.memset(lt[:, W:WPAD], 0.0)
                nc.sync.dma_start(rt[:], right_ap[:, y, :])

                for g in range(NG):
                    ps = psum_pool.tile([128, FT], f32, name=f"ps_{y}_{g}", tag="ps")
                    for j in range(4):
                        t = 4 * g + j
                        q0 = MT * t
                        nc.tensor.matmul(
                            ps[MT * j : MT * (j + 1), :],
                            lhsT=rt[:, q0 : q0 + MT],
                            rhs=lt[:, q0 : q0 + FT],
                            start=True,
                            stop=True,
                            tile_position=(0, MT * j),
                        )
                    st = stage_pool.tile([128, FT], f32, name=f"st_{y}_{g}", tag="st")
                    nc.vector.tensor_copy(st[:], ps[:])
                    nc.sync.dma_start(scr_ap[y, g], st[:])

    nc.compile()
    return nc


def _host_index():
    """idx[d, x] -> flat offset into scr[y] (= [NG*128*FT]) holding G[x-d, x].

    Valid only where x >= d; mask handles the rest.
    """
    d = np.arange(D)[:, None]
    x = np.arange(W)[None, :]
    xp = x - d                       # x' = x - d
    t = np.maximum(xp, 0) // MT      # x'-tile
    q = np.maximum(xp, 0) - MT * t   # row within tile
    g = t // 4
    j = t - 4 * g
    f = x - MT * t                   # col within tile (< 128 always)
    idx = ((g * 128) + (MT * j + q)) * FT + f
    mask = (x >= d)
    return idx.astype(np.int64), mask


def kernel(left, right, num_disparities):
    global _compiled
    left = np.asarray(left)
    right = np.asarray(right)
    assert int(num_disparities) == D
    assert left.shape == (B, C, H, W) and right.shape == (B, C, H, W)

    if _compiled is None:
        _compiled = _build()
    nc = _compiled

    from concourse.bass_utils import run_bass_kernel_spmd

    in_maps = []
    for k in range(8):
        b, hh = k // 2, k % 2
        sl = slice(96 * hh, 96 * hh + 96)
        in_maps.append(
            {
                "left": np.ascontiguousarray(left[b, :, sl, :]),
                "right": np.ascontiguousarray(right[b, :, sl, :]),
            }
        )

    res = run_bass_kernel_spmd(nc, in_maps, list(range(8)))

    idx, mask = _host_index()
    out = np.zeros((B, D, H, W), dtype=np.float32)
    for k in range(8):
        b, hh = k // 2, k % 2
        scr = res.results[k]["scr"].reshape(HSH, -1)   # [96, NG*128*FT]
        gathered = scr[:, idx.ravel()].reshape(HSH, D, W)  # [y, d, x]
        gathered *= mask[None, :, :]
        out[b, :, 96 * hh : 96 * hh + 96, :] = gathered.transpose(1, 0, 2)
    return out
